# revision 41
# baseline (speedup 1.0000x reference)
"""Causal multi-head attention on 8 Trainium2 NeuronCores.

Problem: B=4, T=2048, C=1024, H=16 heads (head_dim 64), causal softmax,
out = softmax(QK^T/8, causal) V projected by Wo, plus bias.

Sharding (hardcoded): 8 cores = 4 batches x 2 head-groups.  Core c handles
batch b = c//2 and heads g*8..g*8+7 where g = c%2 (tensor parallel over
heads: column-split Wq/Wk/Wv, row-split Wo).  Each core returns a partial
output [T, C]; the host sums the two head-group partials per batch and adds
the bias.

Device algorithm (per core), all in "transposed domain" so no on-chip
transposes are needed:
  xT [C, T] arrives head-dim-major (host passes x[b].T).
  qT = Wq_g^T x^T, kT = Wk_g^T x^T   [512, T]  (dims-on-partitions)
  v  = x Wv_g                        [T, 512]  (tokens-on-partitions)
  per head pair, per 512-wide q block, per 128-wide key tile:
    S^T = kT_h^T qT_h  (keys on partitions, two heads row-packed in the
          128x128 PE array via tile_position)
    E = exp(S^T / 8)  on ScalarE (PSUM -> SBUF bf16), causal-masked on the
        diagonal tiles with gpsimd.affine_select
    ctx^T[h] (+= v_tile^T E) via PE with a staggered ones column appended to
        v so that row 64+h of the PSUM tile accumulates sum_keys E = softmax
        denominators.
  denominators are gathered by DMA, inverted on VectorE, broadcast back by
  DMA, and ctx^T is normalized and cast to bf16.
  partial = ctx^T^T Wo_g accumulated over the 4 head-pair K blocks.

Matmuls run in bf16 (inputs rounded once on device); accumulation is fp32
in PSUM.
"""

import numpy as np

import concourse.bass as bass
import concourse.mybir as mybir
import concourse.tile as tile
from concourse.tile import add_dep_helper
from concourse import bacc
from concourse.bass_utils import run_bass_kernel_spmd

F32 = mybir.dt.float32
BF16 = mybir.dt.bfloat16
AF = mybir.ActivationFunctionType

C = 1024
KP = C // 128  # k-tiles over the model dim


def build(S=2048, npair=4):
    """Emit the per-core program.  S = sequence length, npair = head pairs
    (the real problem uses S=2048, npair=4 -> 8 heads, 512 dims per core)."""
    CD = npair * 128        # q/k/v dims owned by this core
    HPC = npair * 2         # heads per core
    NJB = S // 512          # q blocks
    NMT = S // 128          # token tiles

    nc = bacc.Bacc("TRN2", target_bir_lowering=False, debug=False)
    xT = nc.dram_tensor("xT", [C, S], BF16, kind="ExternalInput").ap()
    wq = nc.dram_tensor("wq", [C, CD], BF16, kind="ExternalInput").ap()
    wk = nc.dram_tensor("wk", [C, CD], BF16, kind="ExternalInput").ap()
    wv = nc.dram_tensor("wv", [C, CD], BF16, kind="ExternalInput").ap()
    wo = nc.dram_tensor("wo", [CD, C], BF16, kind="ExternalInput").ap()
    out = nc.dram_tensor("out", [S, C], F32, kind="ExternalOutput").ap()

    with tile.TileContext(nc) as tc:
        # ---- load x and cast to bf16 (staging pool released early) ----
        with tc.tile_pool(name="cpool", bufs=1) as cpool:
            xT_bf = [cpool.tile([128, S], BF16, name=f"xTb{i}", tag=f"xTb{i}")
                     for i in range(KP)]
            wq_bf = [cpool.tile([128, CD], BF16, name=f"wqb{i}", tag=f"wqb{i}")
                     for i in range(KP)]
            wk_bf = [cpool.tile([128, CD], BF16, name=f"wkb{i}", tag=f"wkb{i}")
                     for i in range(KP)]
            wv_bf = [cpool.tile([128, CD], BF16, name=f"wvb{i}", tag=f"wvb{i}")
                     for i in range(KP)]
            wo_bf = [cpool.tile([128, C], BF16, name=f"wob{j}", tag=f"wob{j}")
                     for j in range(npair)]
            # per-(pair, block) tiles: separate tensors prevent Tile from
            # serializing readers of one block behind writers of another
            qT_bf = [[cpool.tile([128, 512], BF16, name=f"qTb{p}_{b}",
                                 tag=f"qTb{p}_{b}") for b in range(NJB)]
                     for p in range(npair)]
            kT_bf = [[cpool.tile([128, 512], BF16, name=f"kTb{p}_{b}",
                                 tag=f"kTb{p}_{b}") for b in range(NJB)]
                     for p in range(npair)]
            # v tiles: per head 65 columns [v_h (64) | staggered ones col],
            # ones column of head h sits at local col 64+h via zero padding:
            # layout per head block of 73 cols: v(64), then onehot(h) over 9.
            VW = 64 + HPC + 1  # per-head block width (<= 73)
            v_bf = [cpool.tile([128, HPC * VW], BF16, name=f"vb{m}", tag=f"vb{m}")
                    for m in range(NMT)]
            ctxT_bf = [[cpool.tile([128, 512], BF16, name=f"cxb{p}_{b}",
                                   tag=f"cxb{p}_{b}") for b in range(NJB)]
                       for p in range(npair)]
            ctxT_f32 = [[cpool.tile([128, 512], F32, name=f"cxf{p}_{b}",
                                    tag=f"cxf{p}_{b}") for b in range(NJB)]
                        for p in range(npair)]

            # inputs arrive pre-rounded to bf16 — DMA straight into the
            # resident tiles, interleaved per k-tile so the first projection
            # matmuls can start as soon as k-tile 0 has landed
            for i in range(KP):
                nc.sync.dma_start(out=xT_bf[i], in_=xT[i * 128:(i + 1) * 128, :])
                nc.scalar.dma_start(out=wq_bf[i], in_=wq[i * 128:(i + 1) * 128, :])
                nc.scalar.dma_start(out=wk_bf[i], in_=wk[i * 128:(i + 1) * 128, :])
                nc.scalar.dma_start(out=wv_bf[i], in_=wv[i * 128:(i + 1) * 128, :])
            # lower-triangle (keep y>=p) bf16 mask for diagonal score tiles
            tri = cpool.tile([128, 128], BF16, name="tri", tag="tri")
            nc.gpsimd.memset(tri, 1.0)
            nc.gpsimd.affine_select(
                out=tri, in_=tri, pattern=[[1, 128]],
                compare_op=mybir.AluOpType.is_ge, fill=0.0, base=0,
                channel_multiplier=-1)
            # v tile init: zero the 9 bookkeeping cols per head block and
            # set the staggered ones column (data cols written by the cast)
            for m in range(NMT):
                vv = v_bf[m].rearrange("p (h x) -> p h x", x=VW)
                nc.gpsimd.memset(vv[:, :, 64:VW], 0.0)
                for h in range(HPC):
                    nc.gpsimd.memset(
                        v_bf[m][:, h * VW + 64 + h:h * VW + 65 + h], 1.0)
            for j in range(npair):
                nc.scalar.dma_start(out=wo_bf[j], in_=wo[j * 128:(j + 1) * 128, :])

            # ---- main pipeline: projections interleaved with attention ----
            with tc.tile_pool(name="psum", bufs=1, space="PSUM") as pp, \
                 tc.tile_pool(name="epool", bufs=6) as epool, \
                 tc.tile_pool(name="srp", bufs=1) as srp, \
                 tc.tile_pool(name="smp", bufs=1) as smp, \
                 tc.tile_pool(name="drp", bufs=4, space="DRAM") as drp, \
                 tc.tile_pool(name="obuf", bufs=2) as obuf, \
                 tc.tile_pool(name="rbp", bufs=3) as rbp:

                attn_last = {}

                def emit_outproj(jb):
                    pinned = False
                    for mt in range(4 * jb, 4 * jb + 4):
                        for nh in range(2):
                            pso = pp.tile([128, 512], F32, name="pso", tag="pj",
                                          bufs=2)
                            for j in range(npair):
                                mm = nc.tensor.matmul(
                                    pso,
                                    lhsT=ctxT_bf[j][mt // 4][
                                        :, (mt % 4) * 128:(mt % 4 + 1) * 128],
                                    rhs=wo_bf[j][:, nh * 512:(nh + 1) * 512],
                                    start=(j == 0), stop=(j == npair - 1))
                                pk = (jb, npair - 1)
                                if not pinned and pk in attn_last:
                                    # keep the scheduler from hoisting this
                                    # block's outproj into its own attention
                                    # (the normalize chain would stall PE)
                                    add_dep_helper(
                                        mm.ins, attn_last[pk].ins,
                                        reason="outproj after own-block attn")
                                    pinned = True
                            ot = obuf.tile([128, 512], F32, name="ot", tag="ot")
                            nc.vector.tensor_copy(ot, pso)
                            nc.sync.dma_start(
                                out=out[mt * 128:(mt + 1) * 128,
                                        nh * 512:(nh + 1) * 512],
                                in_=ot)

                for jb in range(NJB):
                    # -- projections for this token block --
                    for p in range(npair):
                        for (wb, dstT, nm) in ((wq_bf, qT_bf, "pq"),
                                               (wk_bf, kT_bf, "pk")):
                            ps = pp.tile([128, 512], F32, name=nm, tag="pj", bufs=2)
                            for i in range(KP):
                                nc.tensor.matmul(
                                    ps,
                                    lhsT=wb[i][:, p * 128:(p + 1) * 128],
                                    rhs=xT_bf[i][:, jb * 512:(jb + 1) * 512],
                                    start=(i == 0), stop=(i == KP - 1))
                            nc.vector.tensor_copy(dstT[p][jb], ps)
                    for m in range(4 * jb, 4 * jb + 4):
                        psv = pp.tile([128, 512], F32, name="psv", tag="pj", bufs=2)
                        for i in range(KP):
                            nc.tensor.matmul(
                                psv[:, 0:CD],
                                lhsT=xT_bf[i][:, m * 128:(m + 1) * 128],
                                rhs=wv_bf[i],
                                start=(i == 0), stop=(i == KP - 1))
                        vv = v_bf[m].rearrange("p (h x) -> p h x", x=VW)
                        nc.vector.tensor_copy(
                            vv[:, :, 0:64],
                            psv[:, 0:CD].rearrange("p (h x) -> p h x", x=64))

                    # -- attention for this q block --
                    sums_row = srp.tile([VW, 512], F32,
                                        name="sums_row", tag="sr")
                    # 1e-30 (not 0) so unused lanes stay finite under
                    # reciprocal_approx; negligible vs real sums (>= 1)
                    nc.gpsimd.memset(sums_row[64:VW, :], 1e-30)
                    for p in range(npair):
                        h0, h1 = 2 * p, 2 * p + 1
                        c0 = pp.tile([128, 512], F32, name="c0", tag="pc", bufs=2)
                        c1 = pp.tile([128, 512], F32, name="c1", tag="pc", bufs=2)
                        nkt = 4 * (jb + 1)
                        for kt in range(nkt):
                            # causal: keys kt*128.. are only visible to the
                            # q-suffix starting at qoff within this block
                            qoff = max(0, kt * 128 - jb * 512)
                            N = 512 - qoff
                            qs = jb * 512 + qoff
                            ps2 = pp.tile([128, 1024], F32, name="ps2", tag="pa",
                                          bufs=2)
                            kts = kT_bf[p][kt // 4][:, (kt % 4) * 128:
                                                        (kt % 4 + 1) * 128]
                            qts = qT_bf[p][jb][:, qoff:qoff + N]
                            nc.tensor.matmul(
                                ps2[:, 0:N],
                                lhsT=kts[0:64, :], rhs=qts[0:64, :],
                                start=True, stop=True)
                            nc.tensor.matmul(
                                ps2[:, 512:512 + N],
                                lhsT=kts[64:128, :], rhs=qts[64:128, :],
                                start=True, stop=True, tile_position=(64, 0))
                            et = epool.tile([128, 1024], BF16, name="et", tag="et")
                            if qoff == 0:
                                nc.scalar.activation(et, ps2, AF.Exp, scale=0.125)
                            else:
                                nc.scalar.activation(et[:, 0:N], ps2[:, 0:N],
                                                     AF.Exp, scale=0.125)
                                nc.scalar.activation(et[:, 512:512 + N],
                                                     ps2[:, 512:512 + N],
                                                     AF.Exp, scale=0.125)
                            if kt * 128 >= jb * 512:
                                # triangle mask on the first 128 suffix cols
                                for hh in range(2):
                                    nc.vector.tensor_mul(
                                        et[:, hh * 512:hh * 512 + 128],
                                        et[:, hh * 512:hh * 512 + 128],
                                        tri)
                            nc.tensor.matmul(
                                c0[0:VW, qoff:512],
                                lhsT=v_bf[kt][:, h0 * VW:(h0 + 1) * VW],
                                rhs=et[:, 0:N],
                                start=(kt == 0), stop=(kt == nkt - 1))
                            attn_last[(jb, p)] = nc.tensor.matmul(
                                c1[0:VW, qoff:512],
                                lhsT=v_bf[kt][:, h1 * VW:(h1 + 1) * VW],
                                rhs=et[:, 512:512 + N],
                                start=(kt == 0), stop=(kt == nkt - 1))
                        # denominators: rows 64..72 of c0/c1 hold
                        # [0..,sum_h@64+h,..0] (staggered ones cols), so they
                        # compose by addition into one [9, 512] block
                        nc.vector.tensor_add(
                            sums_row[64:VW, :], sums_row[64:VW, :], c0[64:VW, :])
                        nc.vector.tensor_add(
                            sums_row[64:VW, :], sums_row[64:VW, :], c1[64:VW, :])
                        # per-pair normalization: fast DVE reciprocal
                        # (~18 bits, far below the bf16 noise floor).  For the
                        # block's last pair the reciprocal goes first (its DMA
                        # round-trip is the exposed tail); otherwise the psum
                        # copies go first so the ctx banks free sooner.
                        def do_recip():
                            # custom DVE ops only behave at partition base 0 on
                            # hardware: stage the sums rows down with an
                            # aligned copy, then run the fast reciprocal there
                            s8 = smp.tile([HPC, 512], F32, name="s8", tag="s8")
                            nc.vector.tensor_copy(s8, sums_row[64:64 + HPC, :])
                            rr8 = smp.tile([HPC, 512], F32, name="rr8", tag="r8")
                            nc.vector.reciprocal_approx_fast(out=rr8, in_=s8)
                            rd = drp.tile([2, 512], F32, name="rd", tag="rd")
                            nc.sync.dma_start(
                                out=rd, in_=rr8[2 * p:2 * p + 2, :])
                            r64 = rbp.tile([128, 512], F32, name="r64",
                                           tag="r64")
                            bsrc = bass.AP(rd.tensor, rd.offset,
                                           [[512, 2], [0, 64], [1, 512]])
                            nc.sync.dma_start(out=r64, in_=bsrc)
                            return r64

                        def do_copies():
                            nc.vector.tensor_copy(ctxT_f32[p][jb][0:64, :],
                                                  c0[0:64, :])
                            nc.vector.tensor_copy(ctxT_f32[p][jb][64:128, :],
                                                  c1[0:64, :])

                        if p == npair - 1:
                            r64 = do_recip()
                            do_copies()
                        else:
                            do_copies()
                            r64 = do_recip()
                        nc.gpsimd.tensor_mul(
                            ctxT_bf[p][jb], ctxT_f32[p][jb], r64)

                    # output projection for the PREVIOUS block, emitted after
                    # this block's attention so its PSUM stores sit behind the
                    # normalization DMAs in every queue's static order
                    if jb > 0:
                        emit_outproj(jb - 1)

                # trailing output projection for the last block
                emit_outproj(NJB - 1)

    nc.compile()
    return nc


_NC_CACHE = {}


def _get_nc(S=2048, npair=4):
    key = (S, npair)
    if key not in _NC_CACHE:
        _NC_CACHE[key] = build(S, npair)
    return _NC_CACHE[key]


def make_in_maps(x, Wq, Wk, Wv, Wo):
    """Host-side sharding: batch x head-group slices, x transposed to
    dims-major layout, rounded to bf16 (the dtype the device matmuls use)."""
    import ml_dtypes

    bf = ml_dtypes.bfloat16
    in_maps = []
    for c in range(8):
        b, g = divmod(c, 2)
        sl = slice(g * 512, (g + 1) * 512)
        in_maps.append({
            "xT": np.ascontiguousarray(x[b].T).astype(bf),
            "wq": np.ascontiguousarray(Wq[:, sl]).astype(bf),
            "wk": np.ascontiguousarray(Wk[:, sl]).astype(bf),
            "wv": np.ascontiguousarray(Wv[:, sl]).astype(bf),
            "wo": np.ascontiguousarray(Wo[sl, :]).astype(bf),
        })
    return in_maps


def run_cores(x, Wq, Wk, Wv, Wo, trace=False, trace_kwargs=None):
    nc = _get_nc(2048, 4)
    in_maps = make_in_maps(x, Wq, Wk, Wv, Wo)
    return run_bass_kernel_spmd(
        nc, in_maps, core_ids=list(range(8)), trace=trace,
        trace_kwargs=trace_kwargs or {})


def kernel(x, Wq, Wk, Wv, Wo, bo):
    x = np.asarray(x, dtype=np.float32)
    Wq = np.asarray(Wq, dtype=np.float32)
    Wk = np.asarray(Wk, dtype=np.float32)
    Wv = np.asarray(Wv, dtype=np.float32)
    Wo = np.asarray(Wo, dtype=np.float32)
    bo = np.asarray(bo, dtype=np.float32)

    res = run_cores(x, Wq, Wk, Wv, Wo).results
    out = np.empty((4, 2048, 1024), dtype=np.float32)
    for b in range(4):
        out[b] = res[2 * b]["out"] + res[2 * b + 1]["out"] + bo[None, :]
    return out
